# revision 26
# baseline (speedup 1.0000x reference)
"""Trainium2 Bass kernel for nn_ActorCritic (8-core SPMD, data-parallel over edges).

Layout strategy:
  - positions n = e*16+s flattened; each core gets 6250 edges (100k positions),
    padded to 6272 edges / 100352 positions = NT tiles of 512 positions.
  - MLP runs feature-major: activations [features-on-partitions, 512 positions],
    float32r matmuls (full rate at N=512).
  - Per-position scalar math runs position-major after a TensorE transpose of the
    [19, 512] block (16 logits, critic val, u, action) into [128, 19] chunks.
  - Per-graph segment sum: masked vals go to DRAM position-major, get reloaded as
    [128 edge, 16 sample] tiles and matmul'd against a host-built one-hot
    indicator [128 edge, 64 graph]; host adds the 8 per-core partials.
"""

import sys
import types

import ml_dtypes
import numpy as np

sys.path.insert(0, "/opt/trn_rl_repo")

# bass_utils imports antenv.axon_hooks (absent in this image) when trace=True;
# provide a stub so the import never fails. test harnesses may overwrite the
# hook with a real NTFF profiler.
if "antenv.axon_hooks" not in sys.modules:
    _hm = types.ModuleType("antenv.axon_hooks")
    _holder = {"hook": None}
    _hm.set_axon_ntff_profile_hook = lambda h: _holder.__setitem__("hook", h)
    _hm.get_axon_ntff_profile_hook = lambda: _holder["hook"]
    sys.modules["antenv.axon_hooks"] = _hm
    try:
        import antenv

        antenv.axon_hooks = _hm
    except ImportError:
        pass

import concourse.bass as bass
import concourse.tile as tile
from concourse import bacc, mybir

# Force every activation onto the one table that covers Relu/Identity/Exp/Ln
# (avoids ~1.3us ACT_TABLE_LOAD stalls between function-set switches).
_orig_get_act_tables = bacc.get_activation_tables


def _single_act_table(arch):
    tabs = dict(_orig_get_act_tables(arch))
    keep = "natural_log_exp_and_others"
    if keep in tabs:
        tabs = {k: (v if k == keep else set()) for k, v in tabs.items()}
    return tabs


bacc.get_activation_tables = _single_act_table
from concourse.bass_utils import run_bass_kernel_spmd
from concourse.masks import make_identity

F32 = mybir.dt.float32
F32R = mybir.dt.float32r
BF16 = mybir.dt.bfloat16
U8 = mybir.dt.uint8
AF = mybir.ActivationFunctionType
ALU = mybir.AluOpType
AX = mybir.AxisListType

E, S, G, H, O = 50000, 16, 64, 256, 16
NCORES = 8
NE = E // NCORES          # 6250 edges per core
NEP = 6272                # padded edges per core (49 * 128)
NPOS = NEP * S            # 100352 positions per core
T = 512                   # positions per tile
NT = NPOS // T            # 196 tiles
NCH = NEP // 128          # 49 phase-2 chunks


def build_kernel(nt=NT, nch=NCH):
    npos = nt * T
    nep = npos // S
    nc = bacc.Bacc(None, target_bir_lowering=False)

    h_d = nc.declare_dram_parameter("h", [2, npos], BF16, isOutput=False)
    u_d = nc.declare_dram_parameter("u", [1, npos], F32, isOutput=False)
    a_d = nc.declare_dram_parameter("act", [1, npos], F32, isOutput=False)
    w0_d = nc.declare_dram_parameter("w0", [2, 512], BF16, isOutput=False)
    b0_d = nc.declare_dram_parameter("b0", [512], F32, isOutput=False)
    w1a_d = nc.declare_dram_parameter("w1a", [256, 256], BF16, isOutput=False)
    w1c_d = nc.declare_dram_parameter("w1c", [256, 256], BF16, isOutput=False)
    b1_d = nc.declare_dram_parameter("b1", [512], F32, isOutput=False)
    w2_d = nc.declare_dram_parameter("w2", [512, 17], BF16, isOutput=False)
    b2_d = nc.declare_dram_parameter("b2", [17], F32, isOutput=False)
    ind_d = nc.declare_dram_parameter("ind", [nep, 64], F32, isOutput=False)
    iota_d = nc.declare_dram_parameter("iota", [1, 16], F32, isOutput=False)

    alp_d = nc.declare_dram_parameter("alp", [128, nt * 4], F32, isOutput=True)
    msk_d = nc.declare_dram_parameter("msk", [128, nt * 4], U8, isOutput=True)
    vp_d = nc.declare_dram_parameter("vp", [64, 16], F32, isOutput=True)
    es_d = nc.declare_dram_parameter("es", [1, 2], F32, isOutput=True)

    with tile.TileContext(nc) as tc:
        with (
            tc.tile_pool(name="consts", bufs=1) as consts,
            tc.tile_pool(name="wide", bufs=1) as wide,
            tc.tile_pool(name="hin", bufs=3) as hin,
            tc.tile_pool(name="x1", bufs=2) as x1p,
            tc.tile_pool(name="x2", bufs=2) as x2p,
            tc.tile_pool(name="stg", bufs=3) as stg,
            tc.tile_pool(name="pm", bufs=3) as pm,
            tc.tile_pool(name="ph2", bufs=5) as ph2,
            tc.tile_pool(name="ps1", bufs=2, space="PSUM") as ps1,
            tc.tile_pool(name="ps2", bufs=2, space="PSUM") as ps2,
            tc.tile_pool(name="ps3", bufs=2, space="PSUM") as ps3,
            tc.tile_pool(name="pst", bufs=1, space="PSUM") as pst,
            tc.tile_pool(name="psv", bufs=1, space="PSUM") as psv,
        ):
            # ---- constants ----
            w0_t = consts.tile([2, 512], BF16)
            nc.sync.dma_start(out=w0_t[:], in_=w0_d.ap())
            w1a_t = consts.tile([128, 2, 256], BF16)
            nc.sync.dma_start(
                out=w1a_t[:], in_=w1a_d.ap().rearrange("(kc p) m -> p kc m", p=128)
            )
            w1c_t = consts.tile([128, 2, 256], BF16)
            nc.sync.dma_start(
                out=w1c_t[:], in_=w1c_d.ap().rearrange("(kc p) m -> p kc m", p=128)
            )
            w2_t = consts.tile([128, 4, 17], BF16)
            nc.sync.dma_start(
                out=w2_t[:], in_=w2_d.ap().rearrange("(kc p) m -> p kc m", p=128)
            )
            b0_t = consts.tile([128, 4], F32)
            nc.sync.dma_start(
                out=b0_t[:], in_=b0_d.ap().rearrange("(c p) -> p c", p=128)
            )
            b1_t = consts.tile([128, 4], F32)  # cols: a0 a1 c0 c1
            nc.sync.dma_start(
                out=b1_t[:], in_=b1_d.ap().rearrange("(c p) -> p c", p=128)
            )
            b2_t = consts.tile([17, 1], F32)
            nc.sync.dma_start(
                out=b2_t[:], in_=b2_d.ap().rearrange("(p o) -> p o", o=1)
            )
            iota_t = consts.tile([128, 4, 16], F32)
            nc.sync.dma_start(
                out=iota_t[:],
                in_=bass.AP(
                    tensor=iota_d.ap().tensor,
                    offset=0,
                    ap=[[0, 128], [0, 4], [1, 16]],
                ),
            )
            ident = consts.tile([32, 32], F32)
            make_identity(nc, ident[:])
            ones_t = consts.tile([128, 1], F32)
            nc.vector.memset(ones_t[:], 1.0)
            ind_all = consts.tile([128, nch, 64], F32)
            nc.sync.dma_start(
                out=ind_all[:],
                in_=ind_d.ap().rearrange("(k p) g -> p k g", p=128),
            )

            # ---- wide per-position accumulators (written in tile-sized slices) ----
            alp_w = wide.tile([128, nt * 4], F32)
            mskf_w = wide.tile([128, nt * 4], F32)
            msk8_w = wide.tile([128, nt * 4], U8)
            ent_w = wide.tile([128, nt * 4], F32)
            vals_w = wide.tile([128, nt * 4], F32)

            psV_full = psv.tile([64, 16], F32)
            psV = psV_full[:]

            for t in range(nt):
                sl = slice(t * T, (t + 1) * T)
                csl = slice(t * 4, (t + 1) * 4)

                h_t = hin.tile([2, T], BF16)
                nc.sync.dma_start(out=h_t[:], in_=h_d.ap()[:, sl])

                # ---- L1: [2 x T] -> [512 x T] (actor 0:256, critic 256:512) ----
                x1a = x1p.tile([128, 2, T], BF16)
                x1c = x1p.tile([128, 2, T], BF16)
                for mc in range(4):
                    psA = ps1.tile([128, T], F32)
                    nc.tensor.matmul(
                        psA[:], w0_t[:, mc * 128 : (mc + 1) * 128], h_t[:],
                        start=True, stop=True,
                    )
                    dst = x1a[:, mc, :] if mc < 2 else x1c[:, mc - 2, :]
                    if mc % 2 == 0:
                        nc.scalar.activation(
                            out=dst, in_=psA[:], func=AF.Relu,
                            bias=b0_t[:, mc : mc + 1],
                        )
                    else:
                        nc.vector.tensor_scalar(
                            out=dst, in0=psA[:],
                            scalar1=b0_t[:, mc : mc + 1], scalar2=0.0,
                            op0=ALU.add, op1=ALU.max,
                        )

                # ---- L2 per net: [256 x T] -> [256 x T] ----
                x2a = x2p.tile([128, 2, T], BF16)
                x2c = x2p.tile([128, 2, T], BF16)
                for net, (w1_t, x1_t, x2_t, boff) in enumerate(
                    ((w1a_t, x1a, x2a, 0), (w1c_t, x1c, x2c, 2))
                ):
                    for mc in range(2):
                        psB = ps2.tile([128, T], F32)
                        for kc in range(2):
                            nc.tensor.matmul(
                                psB[:],
                                w1_t[:, kc, mc * 128 : (mc + 1) * 128],
                                x1_t[:, kc, :],
                                start=(kc == 0), stop=(kc == 1),
                            )
                        dst = x2_t[:, mc, :]
                        if mc == 0:
                            nc.scalar.activation(
                                out=dst, in_=psB[:], func=AF.Relu,
                                bias=b1_t[:, boff + mc : boff + mc + 1],
                            )
                        else:
                            nc.vector.tensor_scalar(
                                out=dst, in0=psB[:],
                                scalar1=b1_t[:, boff + mc : boff + mc + 1],
                                scalar2=0.0, op0=ALU.add, op1=ALU.max,
                            )

                # ---- L3: blockdiag [512 x 17] -> [17 x T] (16 logits + val) ----
                psC = ps3.tile([17, T], F32)
                rhs_list = (x2a[:, 0, :], x2a[:, 1, :], x2c[:, 0, :], x2c[:, 1, :])
                for kc in range(4):
                    nc.tensor.matmul(
                        psC[:], w2_t[:, kc, :], rhs_list[kc],
                        start=(kc == 0), stop=(kc == 3),
                    )

                # ---- staging [19 x T]: 0:16 logits, 16 val, 17 u, 18 action ----
                stg_t = stg.tile([19, T], F32)
                nc.scalar.activation(
                    out=stg_t[0:17, :], in_=psC[:], func=AF.Identity, bias=b2_t[:]
                )
                nc.sync.dma_start(out=stg_t[17:18, :], in_=u_d.ap()[:, sl])
                nc.sync.dma_start(out=stg_t[18:19, :], in_=a_d.ap()[:, sl])

                # ---- transpose 4x [19,128] -> [128,19] into one PSUM bank ----
                psT = pst.tile([128, 4, 19], F32)
                for c in range(4):
                    nc.tensor.transpose(
                        psT[:, c, :],
                        stg_t[:, c * 128 : (c + 1) * 128],
                        ident[0:19, 0:19],
                    )
                pm_t = pm.tile([128, 4, 19], F32)
                nc.vector.tensor_copy(out=pm_t[:], in_=psT[:])

                L = pm_t[:, :, 0:16]
                V = pm_t[:, :, 16:17]
                U = pm_t[:, :, 17:18]
                A = pm_t[:, :, 18:19]

                # ---- softmax pieces (position-major) ----
                exp_t = pm.tile([128, 4, 16], F32)
                nc.scalar.activation(out=exp_t[:], in_=L, func=AF.Exp)
                zw = pm.tile([128, 4, 1], F32)
                nc.vector.reduce_sum(out=zw[:], in_=exp_t[:], axis=AX.X)
                scr1 = pm.tile([128, 4, 16], F32)
                nc.gpsimd.tensor_mul(out=scr1[:], in0=exp_t[:], in1=L)
                wsum = pm.tile([128, 4, 1], F32)
                nc.vector.reduce_sum(out=wsum[:], in_=scr1[:], axis=AX.X)
                eq = pm.tile([128, 4, 16], F32)
                for c in range(4):
                    nc.vector.tensor_scalar(
                        out=eq[:, c, :], in0=iota_t[:, c, :],
                        scalar1=A[:, c, :], scalar2=None, op0=ALU.is_equal,
                    )
                scr2 = pm.tile([128, 4, 16], F32)
                nc.gpsimd.tensor_mul(out=scr2[:], in0=eq[:], in1=L)
                sel = pm.tile([128, 4, 1], F32)
                nc.vector.reduce_sum(out=sel[:], in_=scr2[:], axis=AX.X)
                lse = pm.tile([128, 4, 1], F32)
                nc.scalar.activation(out=lse[:], in_=zw[:], func=AF.Ln)
                rz = pm.tile([128, 4, 1], F32)
                nc.vector.reciprocal(out=rz[:], in_=zw[:])

                went = pm.tile([128, 4, 1], F32)
                nc.gpsimd.tensor_mul(out=went[:], in0=wsum[:], in1=rz[:])
                ent0 = pm.tile([128, 4, 1], F32)
                nc.gpsimd.tensor_sub(out=ent0[:], in0=lse[:], in1=went[:])

                au = pm.tile([128, 4, 1], F32)
                nc.gpsimd.tensor_mul(out=au[:], in0=U, in1=U)
                mskf = mskf_w[:, csl].rearrange("p (c o) -> p c o", o=1)
                nc.vector.tensor_single_scalar(
                    out=mskf, in_=au[:], scalar=1.0, op=ALU.is_lt
                )
                nc.gpsimd.tensor_copy(
                    out=msk8_w[:, csl].rearrange("p (c o) -> p c o", o=1), in_=mskf
                )

                d0 = pm.tile([128, 4, 1], F32)
                nc.gpsimd.tensor_sub(out=d0[:], in0=sel[:], in1=lse[:])
                nc.gpsimd.tensor_mul(
                    out=alp_w[:, csl].rearrange("p (c o) -> p c o", o=1),
                    in0=d0[:], in1=mskf,
                )
                nc.gpsimd.tensor_mul(
                    out=ent_w[:, csl].rearrange("p (c o) -> p c o", o=1),
                    in0=ent0[:], in1=mskf,
                )
                nc.gpsimd.tensor_mul(
                    out=vals_w[:, csl].rearrange("p (c o) -> p c o", o=1),
                    in0=V, in1=mskf,
                )

            # ---- outputs: alp / mask (contiguous partition-major dumps) ----
            nc.sync.dma_start(out=alp_d.ap(), in_=alp_w[:])
            nc.sync.dma_start(out=msk_d.ap(), in_=msk8_w[:])

            # ---- entropy numerator & mask count ----
            red2 = wide.tile([128, 2], F32)
            nc.vector.reduce_sum(out=red2[:, 0:1], in_=ent_w[:], axis=AX.X)
            nc.vector.reduce_sum(out=red2[:, 1:2], in_=mskf_w[:], axis=AX.X)
            psE = ps1.tile([128, T], F32, tag="psA")
            nc.tensor.matmul(
                psE[0:1, 0:2], ones_t[:], red2[:], start=True, stop=True
            )
            es_t = wide.tile([1, 2], F32)
            nc.vector.tensor_copy(out=es_t[:], in_=psE[0:1, 0:2])
            nc.sync.dma_start(out=es_d.ap(), in_=es_t[:])

            # ---- per-graph segment sum straight out of SBUF ----
            # s-major position order makes vals_w col j = nch*s + k, so the
            # [128 edge, 16 sample] matmul rhs is a stride-nch SBUF view.
            vw_ks = vals_w[:].rearrange("p (s k) -> p k s", k=nch)
            for k in range(nch):
                nc.tensor.matmul(
                    psV, ind_all[:, k, :], vw_ks[:, k, :],
                    start=(k == 0), stop=(k == nch - 1),
                )
            vp_t = ph2.tile([64, 16], F32)
            nc.vector.tensor_copy(out=vp_t[:], in_=psV)
            nc.sync.dma_start(out=vp_d.ap(), in_=vp_t[:])

    nc.compile()
    return nc


def prep_inputs(ob, action, graph_ids, aW0, ab0, aW1, ab1, aW2, ab2,
                cW0, cb0, cW1, cb1, cW2, cb2, nt=NT, nch=NCH):
    npos = nt * T
    nep = npos // S
    ne = min(NE, nep)

    obf = np.ascontiguousarray(np.asarray(ob, dtype=np.float32).reshape(-1, 3))
    actf = np.asarray(action).astype(np.float32).reshape(-1)
    gid = np.asarray(graph_ids).astype(np.int64)

    w0 = np.concatenate([np.asarray(aW0), np.asarray(cW0)], axis=1).astype(ml_dtypes.bfloat16)
    b0 = np.concatenate([np.asarray(ab0), np.asarray(cb0)]).astype(np.float32)
    w1a = np.asarray(aW1).astype(ml_dtypes.bfloat16)
    w1c = np.asarray(cW1).astype(ml_dtypes.bfloat16)
    b1 = np.concatenate([np.asarray(ab1), np.asarray(cb1)]).astype(np.float32)
    w2 = np.zeros((512, 17), ml_dtypes.bfloat16)
    w2[0:256, 0:16] = np.asarray(aW2).astype(ml_dtypes.bfloat16)
    w2[256:512, 16:17] = np.asarray(cW2).astype(ml_dtypes.bfloat16)
    b2 = np.concatenate([np.asarray(ab2), np.asarray(cb2)]).astype(np.float32)
    iota = np.arange(16, dtype=np.float32).reshape(1, 16)

    in_maps = []
    for i in range(NCORES):
        lo = i * NE
        sl = slice(lo * S, (lo + ne) * S)
        ob3 = obf[sl].reshape(ne, S, 3)
        # s-major layout: position n = s * nep + e
        h = np.zeros((2, S, nep), ml_dtypes.bfloat16)
        h[0, :, :ne] = ob3[:, :, 1].T.astype(ml_dtypes.bfloat16)
        h[1, :, :ne] = ob3[:, :, 2].T.astype(ml_dtypes.bfloat16)
        h = h.reshape(2, npos)
        u = np.full((S, nep), 5.0, np.float32)
        u[:, :ne] = ob3[:, :, 0].T
        u = u.reshape(1, npos)
        a = np.zeros((S, nep), np.float32)
        a[:, :ne] = actf[sl].reshape(ne, S).T
        a = a.reshape(1, npos)
        ind = np.zeros((nep, 64), np.float32)
        ind[np.arange(ne), gid[lo : lo + ne]] = 1.0
        in_maps.append({
            "h": h, "u": u, "act": a, "w0": w0, "b0": b0, "w1a": w1a,
            "w1c": w1c, "b1": b1, "w2": w2, "b2": b2, "ind": ind, "iota": iota,
        })
    return in_maps


_cached_nc = None


def kernel(**inputs):
    global _cached_nc
    if _cached_nc is None:
        _cached_nc = build_kernel()
    in_maps = prep_inputs(**inputs)
    res = run_bass_kernel_spmd(_cached_nc, in_maps, core_ids=list(range(NCORES)))

    alp = np.empty((E, S), np.float32)
    mask = np.empty((E, S), np.uint8)
    vp = np.zeros((64, 16), np.float64)
    ent_num = 0.0
    ent_den = 0.0
    for i, r in enumerate(res.results):
        esl = slice(i * NE, (i + 1) * NE)
        # [128, nt*4] partition-major -> flat n = s*NEP + e -> [NE, S]
        alp[esl] = r["alp"].T.reshape(S, NEP)[:, :NE].T
        mask[esl] = r["msk"].T.reshape(S, NEP)[:, :NE].T
        vp += r["vp"].astype(np.float64)
        ent_num += float(r["es"][0, 0])
        ent_den += float(r["es"][0, 1])

    action_log_probs = alp
    avg_entropy = np.float32(ent_num / ent_den)
    value_preds = (vp / E).astype(np.float32)
    edge_mask = mask.astype(bool)
    return action_log_probs, avg_entropy, value_preds, edge_mask


# revision 27
# speedup vs baseline: 1.0485x; 1.0485x over previous
"""Trainium2 Bass kernel for nn_ActorCritic (8-core SPMD, data-parallel over edges).

Layout strategy:
  - positions n = e*16+s flattened; each core gets 6250 edges (100k positions),
    padded to 6272 edges / 100352 positions = NT tiles of 512 positions.
  - MLP runs feature-major: activations [features-on-partitions, 512 positions],
    float32r matmuls (full rate at N=512).
  - Per-position scalar math runs position-major after a TensorE transpose of the
    [19, 512] block (16 logits, critic val, u, action) into [128, 19] chunks.
  - Per-graph segment sum: masked vals go to DRAM position-major, get reloaded as
    [128 edge, 16 sample] tiles and matmul'd against a host-built one-hot
    indicator [128 edge, 64 graph]; host adds the 8 per-core partials.
"""

import sys
import types

import ml_dtypes
import numpy as np

sys.path.insert(0, "/opt/trn_rl_repo")

# bass_utils imports antenv.axon_hooks (absent in this image) when trace=True;
# provide a stub so the import never fails. test harnesses may overwrite the
# hook with a real NTFF profiler.
if "antenv.axon_hooks" not in sys.modules:
    _hm = types.ModuleType("antenv.axon_hooks")
    _holder = {"hook": None}
    _hm.set_axon_ntff_profile_hook = lambda h: _holder.__setitem__("hook", h)
    _hm.get_axon_ntff_profile_hook = lambda: _holder["hook"]
    sys.modules["antenv.axon_hooks"] = _hm
    try:
        import antenv

        antenv.axon_hooks = _hm
    except ImportError:
        pass

import concourse.bass as bass
import concourse.tile as tile
from concourse import bacc, mybir

# Force every activation onto the one table that covers Relu/Identity/Exp/Ln
# (avoids ~1.3us ACT_TABLE_LOAD stalls between function-set switches).
_orig_get_act_tables = bacc.get_activation_tables


def _single_act_table(arch):
    tabs = dict(_orig_get_act_tables(arch))
    keep = "natural_log_exp_and_others"
    if keep in tabs:
        tabs = {k: (v if k == keep else set()) for k, v in tabs.items()}
    return tabs


bacc.get_activation_tables = _single_act_table
from concourse.bass_utils import run_bass_kernel_spmd
from concourse.masks import make_identity

F32 = mybir.dt.float32
F32R = mybir.dt.float32r
BF16 = mybir.dt.bfloat16
U8 = mybir.dt.uint8
AF = mybir.ActivationFunctionType
ALU = mybir.AluOpType
AX = mybir.AxisListType

E, S, G, H, O = 50000, 16, 64, 256, 16
NCORES = 8
NE = E // NCORES          # 6250 edges per core
NEP = 6272                # padded edges per core (49 * 128)
NPOS = NEP * S            # 100352 positions per core
T = 512                   # positions per tile
NT = NPOS // T            # 196 tiles
NCH = NEP // 128          # 49 phase-2 chunks


def build_kernel(nt=NT, nch=NCH):
    npos = nt * T
    nep = npos // S
    nc = bacc.Bacc(None, target_bir_lowering=False)

    h_d = nc.declare_dram_parameter("h", [2, npos], BF16, isOutput=False)
    u_d = nc.declare_dram_parameter("u", [1, npos], F32, isOutput=False)
    a_d = nc.declare_dram_parameter("act", [1, npos], F32, isOutput=False)
    w0_d = nc.declare_dram_parameter("w0", [2, 512], BF16, isOutput=False)
    b0_d = nc.declare_dram_parameter("b0", [512], F32, isOutput=False)
    w1a_d = nc.declare_dram_parameter("w1a", [256, 256], BF16, isOutput=False)
    w1c_d = nc.declare_dram_parameter("w1c", [256, 256], BF16, isOutput=False)
    b1_d = nc.declare_dram_parameter("b1", [512], F32, isOutput=False)
    w2_d = nc.declare_dram_parameter("w2", [512, 17], BF16, isOutput=False)
    b2_d = nc.declare_dram_parameter("b2", [17], F32, isOutput=False)
    ind_d = nc.declare_dram_parameter("ind", [nep, 64], F32, isOutput=False)
    iota_d = nc.declare_dram_parameter("iota", [1, 16], F32, isOutput=False)

    alp_d = nc.declare_dram_parameter("alp", [128, nt * 4], F32, isOutput=True)
    msk_d = nc.declare_dram_parameter("msk", [128, nt * 4], U8, isOutput=True)
    vp_d = nc.declare_dram_parameter("vp", [64, 16], F32, isOutput=True)
    es_d = nc.declare_dram_parameter("es", [1, 2], F32, isOutput=True)

    with tile.TileContext(nc) as tc:
        with (
            tc.tile_pool(name="consts", bufs=1) as consts,
            tc.tile_pool(name="wide", bufs=1) as wide,
            tc.tile_pool(name="hin", bufs=3) as hin,
            tc.tile_pool(name="x1", bufs=2) as x1p,
            tc.tile_pool(name="x2", bufs=2) as x2p,
            tc.tile_pool(name="stg", bufs=3) as stg,
            tc.tile_pool(name="pm", bufs=4) as pm,
            tc.tile_pool(name="ph2", bufs=5) as ph2,
            tc.tile_pool(name="ps1", bufs=2, space="PSUM") as ps1,
            tc.tile_pool(name="ps2", bufs=2, space="PSUM") as ps2,
            tc.tile_pool(name="ps3", bufs=2, space="PSUM") as ps3,
            tc.tile_pool(name="pst", bufs=1, space="PSUM") as pst,
            tc.tile_pool(name="psv", bufs=1, space="PSUM") as psv,
        ):
            # ---- constants ----
            w0_t = consts.tile([2, 512], BF16)
            nc.sync.dma_start(out=w0_t[:], in_=w0_d.ap())
            w1a_t = consts.tile([128, 2, 256], BF16)
            nc.sync.dma_start(
                out=w1a_t[:], in_=w1a_d.ap().rearrange("(kc p) m -> p kc m", p=128)
            )
            w1c_t = consts.tile([128, 2, 256], BF16)
            nc.sync.dma_start(
                out=w1c_t[:], in_=w1c_d.ap().rearrange("(kc p) m -> p kc m", p=128)
            )
            w2_t = consts.tile([128, 4, 17], BF16)
            nc.sync.dma_start(
                out=w2_t[:], in_=w2_d.ap().rearrange("(kc p) m -> p kc m", p=128)
            )
            b0_t = consts.tile([128, 4], F32)
            nc.sync.dma_start(
                out=b0_t[:], in_=b0_d.ap().rearrange("(c p) -> p c", p=128)
            )
            b1_t = consts.tile([128, 4], F32)  # cols: a0 a1 c0 c1
            nc.sync.dma_start(
                out=b1_t[:], in_=b1_d.ap().rearrange("(c p) -> p c", p=128)
            )
            b2_t = consts.tile([17, 1], F32)
            nc.sync.dma_start(
                out=b2_t[:], in_=b2_d.ap().rearrange("(p o) -> p o", o=1)
            )
            iota_t = consts.tile([128, 4, 16], F32)
            nc.sync.dma_start(
                out=iota_t[:],
                in_=bass.AP(
                    tensor=iota_d.ap().tensor,
                    offset=0,
                    ap=[[0, 128], [0, 4], [1, 16]],
                ),
            )
            ident = consts.tile([32, 32], F32)
            make_identity(nc, ident[:])
            ones_t = consts.tile([128, 1], F32)
            nc.vector.memset(ones_t[:], 1.0)

            # ---- wide per-position accumulators (written in tile-sized slices) ----
            alp_w = wide.tile([128, nt * 4], F32)
            mskf_w = wide.tile([128, nt * 4], F32)
            msk8_w = wide.tile([128, nt * 4], U8)
            ent_w = wide.tile([128, nt * 4], F32)
            vals_w = wide.tile([128, nt * 4], F32)

            psV_full = psv.tile([64, 16], F32)
            psV = psV_full[:]

            for t in range(nt):
                sl = slice(t * T, (t + 1) * T)
                csl = slice(t * 4, (t + 1) * 4)

                h_t = hin.tile([2, T], BF16)
                nc.sync.dma_start(out=h_t[:], in_=h_d.ap()[:, sl])

                # ---- L1: [2 x T] -> [512 x T] (actor 0:256, critic 256:512) ----
                x1a = x1p.tile([128, 2, T], BF16)
                x1c = x1p.tile([128, 2, T], BF16)
                for mc in range(4):
                    psA = ps1.tile([128, T], F32)
                    nc.tensor.matmul(
                        psA[:], w0_t[:, mc * 128 : (mc + 1) * 128], h_t[:],
                        start=True, stop=True,
                    )
                    dst = x1a[:, mc, :] if mc < 2 else x1c[:, mc - 2, :]
                    if mc % 2 == 0:
                        nc.scalar.activation(
                            out=dst, in_=psA[:], func=AF.Relu,
                            bias=b0_t[:, mc : mc + 1],
                        )
                    else:
                        nc.vector.tensor_scalar(
                            out=dst, in0=psA[:],
                            scalar1=b0_t[:, mc : mc + 1], scalar2=0.0,
                            op0=ALU.add, op1=ALU.max,
                        )

                # ---- L2 per net: [256 x T] -> [256 x T] ----
                x2a = x2p.tile([128, 2, T], BF16)
                x2c = x2p.tile([128, 2, T], BF16)
                for net, (w1_t, x1_t, x2_t, boff) in enumerate(
                    ((w1a_t, x1a, x2a, 0), (w1c_t, x1c, x2c, 2))
                ):
                    for mc in range(2):
                        psB = ps2.tile([128, T], F32)
                        for kc in range(2):
                            nc.tensor.matmul(
                                psB[:],
                                w1_t[:, kc, mc * 128 : (mc + 1) * 128],
                                x1_t[:, kc, :],
                                start=(kc == 0), stop=(kc == 1),
                            )
                        dst = x2_t[:, mc, :]
                        if mc == 0:
                            nc.scalar.activation(
                                out=dst, in_=psB[:], func=AF.Relu,
                                bias=b1_t[:, boff + mc : boff + mc + 1],
                            )
                        else:
                            nc.vector.tensor_scalar(
                                out=dst, in0=psB[:],
                                scalar1=b1_t[:, boff + mc : boff + mc + 1],
                                scalar2=0.0, op0=ALU.add, op1=ALU.max,
                            )

                # ---- L3: blockdiag [512 x 17] -> [17 x T] (16 logits + val) ----
                psC = ps3.tile([17, T], F32)
                rhs_list = (x2a[:, 0, :], x2a[:, 1, :], x2c[:, 0, :], x2c[:, 1, :])
                for kc in range(4):
                    nc.tensor.matmul(
                        psC[:], w2_t[:, kc, :], rhs_list[kc],
                        start=(kc == 0), stop=(kc == 3),
                    )

                # ---- staging [19 x T]: 0:16 logits, 16 val, 17 u, 18 action ----
                stg_t = stg.tile([19, T], F32)
                nc.scalar.activation(
                    out=stg_t[0:17, :], in_=psC[:], func=AF.Identity, bias=b2_t[:]
                )
                nc.sync.dma_start(out=stg_t[17:18, :], in_=u_d.ap()[:, sl])
                nc.sync.dma_start(out=stg_t[18:19, :], in_=a_d.ap()[:, sl])

                # ---- transpose 4x [19,128] -> [128,19] into one PSUM bank ----
                psT = pst.tile([128, 4, 19], F32)
                for c in range(4):
                    nc.tensor.transpose(
                        psT[:, c, :],
                        stg_t[:, c * 128 : (c + 1) * 128],
                        ident[0:19, 0:19],
                    )
                pm_t = pm.tile([128, 4, 19], F32)
                nc.vector.tensor_copy(out=pm_t[:], in_=psT[:])

                L = pm_t[:, :, 0:16]
                V = pm_t[:, :, 16:17]
                U = pm_t[:, :, 17:18]
                A = pm_t[:, :, 18:19]

                # ---- softmax pieces (position-major) ----
                cat3 = pm.tile([128, 3, 4, 16], F32)
                exp_t = cat3[:, 0, :, :]
                nc.scalar.activation(out=exp_t, in_=L, func=AF.Exp)
                eq = pm.tile([128, 4, 16], F32)
                for c in range(4):
                    nc.vector.tensor_scalar(
                        out=eq[:, c, :], in0=iota_t[:, c, :],
                        scalar1=A[:, c, :], scalar2=None, op0=ALU.is_equal,
                    )
                nc.gpsimd.tensor_mul(out=cat3[:, 1, :, :], in0=exp_t, in1=L)
                nc.gpsimd.tensor_mul(out=cat3[:, 2, :, :], in0=eq[:], in1=L)
                red3 = pm.tile([128, 3, 4, 1], F32)
                nc.vector.reduce_sum(out=red3[:], in_=cat3[:], axis=AX.X)
                zw = red3[:, 0, :, :]
                wsum = red3[:, 1, :, :]
                sel = red3[:, 2, :, :]
                lse = pm.tile([128, 4, 1], F32)
                nc.scalar.activation(out=lse[:], in_=zw, func=AF.Ln)
                rz = pm.tile([128, 4, 1], F32)
                nc.vector.reciprocal(out=rz[:], in_=zw)

                went = pm.tile([128, 4, 1], F32)
                nc.gpsimd.tensor_mul(out=went[:], in0=wsum, in1=rz[:])
                ent0 = pm.tile([128, 4, 1], F32)
                nc.gpsimd.tensor_sub(out=ent0[:], in0=lse[:], in1=went[:])

                au = pm.tile([128, 4, 1], F32)
                nc.gpsimd.tensor_mul(out=au[:], in0=U, in1=U)
                mskf = mskf_w[:, csl].rearrange("p (c o) -> p c o", o=1)
                nc.vector.tensor_single_scalar(
                    out=mskf, in_=au[:], scalar=1.0, op=ALU.is_lt
                )
                nc.gpsimd.tensor_copy(
                    out=msk8_w[:, csl].rearrange("p (c o) -> p c o", o=1), in_=mskf
                )

                d0 = pm.tile([128, 4, 1], F32)
                nc.gpsimd.tensor_sub(out=d0[:], in0=sel, in1=lse[:])
                nc.gpsimd.tensor_mul(
                    out=alp_w[:, csl].rearrange("p (c o) -> p c o", o=1),
                    in0=d0[:], in1=mskf,
                )
                nc.gpsimd.tensor_mul(
                    out=ent_w[:, csl].rearrange("p (c o) -> p c o", o=1),
                    in0=ent0[:], in1=mskf,
                )
                nc.gpsimd.tensor_mul(
                    out=vals_w[:, csl].rearrange("p (c o) -> p c o", o=1),
                    in0=V, in1=mskf,
                )

            # ---- outputs: alp / mask (contiguous partition-major dumps) ----
            nc.sync.dma_start(out=alp_d.ap(), in_=alp_w[:])
            nc.sync.dma_start(out=msk_d.ap(), in_=msk8_w[:])

            # ---- entropy numerator & mask count ----
            red2 = wide.tile([128, 2], F32)
            nc.vector.reduce_sum(out=red2[:, 0:1], in_=ent_w[:], axis=AX.X)
            nc.vector.reduce_sum(out=red2[:, 1:2], in_=mskf_w[:], axis=AX.X)
            psE = ps1.tile([128, T], F32, tag="psA")
            nc.tensor.matmul(
                psE[0:1, 0:2], ones_t[:], red2[:], start=True, stop=True
            )
            es_t = wide.tile([1, 2], F32)
            nc.vector.tensor_copy(out=es_t[:], in_=psE[0:1, 0:2])
            nc.sync.dma_start(out=es_d.ap(), in_=es_t[:])

            # ---- per-graph segment sum straight out of SBUF ----
            ind_all = consts.tile([128, nch, 64], F32)
            nc.sync.dma_start(
                out=ind_all[:],
                in_=ind_d.ap().rearrange("(k p) g -> p k g", p=128),
            )
            # s-major position order makes vals_w col j = nch*s + k, so the
            # [128 edge, 16 sample] matmul rhs is a stride-nch SBUF view.
            vw_ks = vals_w[:].rearrange("p (s k) -> p k s", k=nch)
            for k in range(nch):
                nc.tensor.matmul(
                    psV, ind_all[:, k, :], vw_ks[:, k, :],
                    start=(k == 0), stop=(k == nch - 1),
                )
            vp_t = ph2.tile([64, 16], F32)
            nc.vector.tensor_copy(out=vp_t[:], in_=psV)
            nc.sync.dma_start(out=vp_d.ap(), in_=vp_t[:])

    nc.compile()
    return nc


def prep_inputs(ob, action, graph_ids, aW0, ab0, aW1, ab1, aW2, ab2,
                cW0, cb0, cW1, cb1, cW2, cb2, nt=NT, nch=NCH):
    npos = nt * T
    nep = npos // S
    ne = min(NE, nep)

    obf = np.ascontiguousarray(np.asarray(ob, dtype=np.float32).reshape(-1, 3))
    actf = np.asarray(action).astype(np.float32).reshape(-1)
    gid = np.asarray(graph_ids).astype(np.int64)

    w0 = np.concatenate([np.asarray(aW0), np.asarray(cW0)], axis=1).astype(ml_dtypes.bfloat16)
    b0 = np.concatenate([np.asarray(ab0), np.asarray(cb0)]).astype(np.float32)
    w1a = np.asarray(aW1).astype(ml_dtypes.bfloat16)
    w1c = np.asarray(cW1).astype(ml_dtypes.bfloat16)
    b1 = np.concatenate([np.asarray(ab1), np.asarray(cb1)]).astype(np.float32)
    w2 = np.zeros((512, 17), ml_dtypes.bfloat16)
    w2[0:256, 0:16] = np.asarray(aW2).astype(ml_dtypes.bfloat16)
    w2[256:512, 16:17] = np.asarray(cW2).astype(ml_dtypes.bfloat16)
    b2 = np.concatenate([np.asarray(ab2), np.asarray(cb2)]).astype(np.float32)
    iota = np.arange(16, dtype=np.float32).reshape(1, 16)

    in_maps = []
    for i in range(NCORES):
        lo = i * NE
        sl = slice(lo * S, (lo + ne) * S)
        ob3 = obf[sl].reshape(ne, S, 3)
        # s-major layout: position n = s * nep + e
        h = np.zeros((2, S, nep), ml_dtypes.bfloat16)
        h[0, :, :ne] = ob3[:, :, 1].T.astype(ml_dtypes.bfloat16)
        h[1, :, :ne] = ob3[:, :, 2].T.astype(ml_dtypes.bfloat16)
        h = h.reshape(2, npos)
        u = np.full((S, nep), 5.0, np.float32)
        u[:, :ne] = ob3[:, :, 0].T
        u = u.reshape(1, npos)
        a = np.zeros((S, nep), np.float32)
        a[:, :ne] = actf[sl].reshape(ne, S).T
        a = a.reshape(1, npos)
        ind = np.zeros((nep, 64), np.float32)
        ind[np.arange(ne), gid[lo : lo + ne]] = 1.0
        in_maps.append({
            "h": h, "u": u, "act": a, "w0": w0, "b0": b0, "w1a": w1a,
            "w1c": w1c, "b1": b1, "w2": w2, "b2": b2, "ind": ind, "iota": iota,
        })
    return in_maps


_cached_nc = None


def kernel(**inputs):
    global _cached_nc
    if _cached_nc is None:
        _cached_nc = build_kernel()
    in_maps = prep_inputs(**inputs)
    res = run_bass_kernel_spmd(_cached_nc, in_maps, core_ids=list(range(NCORES)))

    alp = np.empty((E, S), np.float32)
    mask = np.empty((E, S), np.uint8)
    vp = np.zeros((64, 16), np.float64)
    ent_num = 0.0
    ent_den = 0.0
    for i, r in enumerate(res.results):
        esl = slice(i * NE, (i + 1) * NE)
        # [128, nt*4] partition-major -> flat n = s*NEP + e -> [NE, S]
        alp[esl] = r["alp"].T.reshape(S, NEP)[:, :NE].T
        mask[esl] = r["msk"].T.reshape(S, NEP)[:, :NE].T
        vp += r["vp"].astype(np.float64)
        ent_num += float(r["es"][0, 0])
        ent_den += float(r["es"][0, 1])

    action_log_probs = alp
    avg_entropy = np.float32(ent_num / ent_den)
    value_preds = (vp / E).astype(np.float32)
    edge_mask = mask.astype(bool)
    return action_log_probs, avg_entropy, value_preds, edge_mask
